# revision 1
# baseline (speedup 1.0000x reference)
"""Trainium2 Bass kernel for nn_BCE_Loss (focal-style BCE-with-logits, mean).

Reference math per anchor row x[0:3] (logits) and integer target c:
    col = 0 if c==1 else 1 if c==3 else 2
    t   = one_hot(col, 3)
    w   = (1-pt)^2,  pt = x*t + (1-x)*(1-t)        [from detached logits]
    bce = max(x,0) - x*t + log1p(exp(-|x|))
    out = mean(w * bce)

Identity used on device (exact, per element, t in {0,1}):
    loss = (v + t)^2 * softplus(v),   v = x * (1 - 2t)

The expanded one-hot enters only through g = 0.5 - t (so v = 2*g*x), built
with three tensor_scalar ops straight from the integer targets:
    g[:,0] = (targ != 1) - 0.5
    g[:,1] = (targ != 3) - 0.5
    g[:,2] = (targ % 2)  - 0.5        (targ in 0..4: col2 <=> targ even)

With u := v + t = (2h + 0.5) - g and usq = u^2:
    sum loss = sum usq * sp = diagsum(usq_chunk^T @ sp_chunk over 128-wide chunks)
The elementwise-dot reduction runs on the otherwise-idle TensorE as chunked
128x128 matmuls accumulating into one PSUM bank; the diagonal is extracted
once at the end with an identity mask. softplus has no ACT table set on
this toolchain, so sp = Ln(Exp(v) + 1) (both functions plus Square live in
natural_log_exp_and_others — the _Bacc subclass pins that set so the ACT
tables load exactly once; the +1 rides Ln's bias, the *2 rides Exp's scale).
x is loaded f32->bf16 by the SWDGE cast-DMA (no on-chip convert pass); the
Square runs on ACT except for 2 of 8 tiles where it runs on DVE, balancing
the two near-critical engines (measured: ACT stream ~65us, DVE ~70us
compute; HW exec ~92.5us/core, rel err 8.5e-05, vs ~47us DMA roofline).

Sharding: pure data-parallel across 8 NeuronCores — each core takes a
contiguous block of anchors; per-core output is a single partial sum; the
host sums the 8 partials and divides by the element count.
"""

import numpy as np

import concourse.bacc as bacc
import concourse.bass as bass
import concourse.mybir as mybir
from concourse import bass_utils
from concourse.alu_op_type import AluOpType
from concourse.tile import TileContext

N_CORES = 8
N_ANCHORS = 8388608
N_CLASSES = 3
N_SHARD = N_ANCHORS // N_CORES  # 1048576
P = 128  # SBUF partitions
T = 1024  # anchor rows per partition per tile
NT = N_SHARD // (P * T)  # 8 tiles per core
F = N_CLASSES * T  # free dim of an x tile
MM = 128  # diag-trick matmul chunk width


class _Bacc(bacc.Bacc):
    """Bacc with the ACT table pinned to natural_log_exp_and_others.

    The default chooser puts Exp in exp_and_others and Ln in natural_log,
    reloading tables every tile (~1.3us each). Both live in
    natural_log_exp_and_others; emptying every other set (positions kept —
    act_func_set_id is the index into act_info.json) forces one load."""

    _ACT_SET = "natural_log_exp_and_others"

    def insert_act_table_loads(self):
        import bass_rust as _bass_rust

        from concourse.hw_specs import get_activation_tables

        has_activation = any(
            isinstance(i, mybir.InstActivation)
            for b in self.main_func.blocks
            for i in b.instructions
        )
        if not has_activation:
            return
        tables = [
            (name, (fns if name == self._ACT_SET else set()))
            for name, fns in get_activation_tables(self.m.arch).items()
        ]
        _bass_rust.insert_act_table_loads(self, tables)


def _build_nc(targ_is_int64: bool) -> bass.Bass:
    nc = _Bacc("TRN2", target_bir_lowering=False, num_swdge_queues=4)
    pred = nc.dram_tensor(
        "pred", [N_SHARD, N_CLASSES], mybir.dt.float32, kind="ExternalInput"
    )
    n_targ_words = N_SHARD * (2 if targ_is_int64 else 1)
    targ = nc.dram_tensor("targ32", [n_targ_words], mybir.dt.int32, kind="ExternalInput")
    ident = nc.dram_tensor("ident", [P, P], mybir.dt.bfloat16, kind="ExternalInput")
    out = nc.dram_tensor("out", [1], mybir.dt.float32, kind="ExternalOutput")

    xv = pred.rearrange("(n p t) m -> n p (t m)", p=P, t=T)
    tw = 2 * T if targ_is_int64 else T
    tv = targ.rearrange("(n p t) -> n p t", p=P, t=tw)

    with TileContext(nc) as tc:
        with (
            tc.tile_pool(name="io", bufs=4) as io,
            tc.tile_pool(name="tmp", bufs=3) as tmp,
            tc.tile_pool(name="epool", bufs=3) as epool,
            tc.tile_pool(name="mm", bufs=4) as mmp,
            tc.tile_pool(name="singles", bufs=1) as singles,
            tc.tile_pool(name="psum", bufs=1, space="PSUM") as psum,
        ):
            ones_f = singles.tile([P, 1], mybir.dt.float32)
            nc.vector.memset(ones_f, 1.0)
            psA = psum.tile([P, MM], mybir.dt.float32)

            n_mm = F // MM
            for i in range(NT):
                # x loaded with f32->bf16 cast in the DMA datapath (SWDGE)
                xb = io.tile([P, F], mybir.dt.bfloat16)
                tg = io.tile([P, tw], mybir.dt.int32)
                nc.gpsimd.dma_start(out=xb, in_=xv[i])
                nc.sync.dma_start(out=tg, in_=tv[i])

                # g = 0.5 - t (expanded one-hot), strided per-class writes
                g = tmp.tile([P, F], mybir.dt.bfloat16)
                g3 = g.rearrange("p (t m) -> p t m", m=N_CLASSES)
                if targ_is_int64:
                    tlo = tg.rearrange("p (t two) -> p t two", two=2)[:, :, 0]
                else:
                    tlo = tg
                nc.vector.tensor_scalar(
                    out=g3[:, :, 0], in0=tlo, scalar1=1, scalar2=0.5,
                    op0=AluOpType.not_equal, op1=AluOpType.subtract)
                nc.vector.tensor_scalar(
                    out=g3[:, :, 1], in0=tlo, scalar1=3, scalar2=0.5,
                    op0=AluOpType.not_equal, op1=AluOpType.subtract)
                # g2 = 0.5 - t2 = (0.5 - g0) - g1  (reverse0: scalar - in0)
                ig2 = nc.vector.scalar_tensor_tensor(
                    out=g3[:, :, 2], in0=g3[:, :, 0], scalar=0.5, in1=g3[:, :, 1],
                    op0=AluOpType.subtract, op1=AluOpType.subtract)
                ig2.ins.reverse0 = True

                # h = g * x  (v = 2h)
                h = tmp.tile([P, F], mybir.dt.bfloat16)
                nc.vector.tensor_tensor(out=h, in0=g, in1=xb, op=AluOpType.mult)

                # E = exp(2h) = e^v
                E = epool.tile([P, F], mybir.dt.bfloat16)
                nc.scalar.activation(
                    out=E, in_=h, func=mybir.ActivationFunctionType.Exp, scale=2.0)

                # sp = ln(E + 1) = softplus(v)
                sp = mmp.tile([P, F], mybir.dt.bfloat16)
                nc.scalar.activation(
                    out=sp, in_=E, func=mybir.ActivationFunctionType.Ln, bias=1.0)

                # v05 = v + 0.5 ;  u = v05 - g = v + t ;  usq = u^2 (ACT)
                v05 = tmp.tile([P, F], mybir.dt.bfloat16)
                nc.vector.tensor_scalar(
                    out=v05, in0=h, scalar1=2.0, scalar2=0.5,
                    op0=AluOpType.mult, op1=AluOpType.add)
                u = tmp.tile([P, F], mybir.dt.bfloat16)
                nc.vector.tensor_tensor(out=u, in0=v05, in1=g, op=AluOpType.subtract)
                usq = mmp.tile([P, F], mybir.dt.bfloat16)
                if i % 4 == 3:
                    # keep the ACT stream (the pacer) short: 2 of 8 squares on DVE
                    nc.vector.tensor_tensor(out=usq, in0=u, in1=u, op=AluOpType.mult)
                else:
                    nc.scalar.activation(
                        out=usq, in_=u, func=mybir.ActivationFunctionType.Square)

                # TensorE: psA += usq_c^T @ sp_c ; diag holds sum u^2*sp
                for c in range(n_mm):
                    s = slice(c * MM, (c + 1) * MM)
                    nc.tensor.matmul(
                        psA[:, :], usq[:, s], sp[:, s],
                        start=(i == 0 and c == 0),
                        stop=(i == NT - 1 and c == n_mm - 1))

            # epilogue: total = diagsum(psA)
            id_t = singles.tile([P, P], mybir.dt.bfloat16)
            nc.sync.dma_start(out=id_t, in_=ident[:, :])
            dA = singles.tile([P, P], mybir.dt.float32)
            nc.vector.tensor_tensor(out=dA, in0=psA, in1=id_t, op=AluOpType.mult)
            rA = singles.tile([P, 1], mybir.dt.float32)
            nc.vector.tensor_reduce(
                out=rA, in_=dA, axis=mybir.AxisListType.X, op=AluOpType.add)

            psT = psum.tile([1, 1], mybir.dt.float32)
            nc.tensor.matmul(psT[:, :], ones_f[:, :], rA[:, :], start=True, stop=True)
            res = singles.tile([1, 1], mybir.dt.float32)
            nc.vector.tensor_copy(out=res, in_=psT)
            nc.sync.dma_start(out=out[:], in_=res[0, :])

    nc.compile()
    return nc


_cache: dict[bool, bass.Bass] = {}
last_results = None  # BassKernelResults of the most recent run (for test.py)


def _get_nc(targ_is_int64: bool) -> bass.Bass:
    if targ_is_int64 not in _cache:
        _cache[targ_is_int64] = _build_nc(targ_is_int64)
    return _cache[targ_is_int64]


def _identity_bf16() -> np.ndarray:
    import ml_dtypes

    return np.eye(P, dtype=ml_dtypes.bfloat16)


def kernel(pred: np.ndarray, targ: np.ndarray, *, trace: bool = False) -> np.ndarray:
    global last_results
    pred = np.ascontiguousarray(np.asarray(pred, dtype=np.float32))
    targ = np.asarray(targ)
    assert pred.shape == (N_ANCHORS, N_CLASSES), pred.shape
    assert targ.shape == (N_ANCHORS,), targ.shape

    targ_is_int64 = targ.dtype.itemsize == 8
    if targ_is_int64:
        targ_words = np.ascontiguousarray(targ).view(np.int32)  # [2*N] lo,hi pairs
        words_per_shard = 2 * N_SHARD
    else:
        targ_words = np.ascontiguousarray(targ.astype(np.int32, copy=False))
        words_per_shard = N_SHARD

    nc = _get_nc(targ_is_int64)
    ident = _identity_bf16()

    in_maps = []
    for c in range(N_CORES):
        in_maps.append({
            "pred": pred[c * N_SHARD : (c + 1) * N_SHARD],
            "targ32": targ_words[c * words_per_shard : (c + 1) * words_per_shard],
            "ident": ident,
        })

    res = bass_utils.run_bass_kernel_spmd(
        nc, in_maps, core_ids=list(range(N_CORES)), trace=trace
    )
    last_results = res

    total = np.float64(0.0)
    for r in res.results:
        total += np.float64(r["out"][0])
    mean = total / (N_ANCHORS * N_CLASSES)
    return np.float32(mean)

